# revision 6
# baseline (speedup 1.0000x reference)
"""Space-to-depth (8x8 chessboard) kernel for Trainium2.

Full input  : (32, 256, 256, 32) f32
Full output : (32, 8, 8, 32768) f32
out[b, i, j] = inputs[b, i*32:(i+1)*32, j*32:(j+1)*32, :].reshape(-1)

Sharding: batch dim (32) split across 8 NeuronCores (pure data parallel,
no communication) -> 4 examples per core.

Per core the op is pure HBM->HBM data movement, done entirely with DMA
access patterns (no compute engines). Within one (example b, 32-row
band i), iterating (r, j, elem) makes the source AP contiguous and the
destination a 3D AP, so a single DMA moves a block of rows in 4 KiB
contiguous chunks:

  src [[8192, nr], [1024, k], [1, 1024]]   (contiguous 32 KiB per row r)
  dst [[1024, nr], [32768, k], [1, 1024]]  (4 KiB chunks, 32 KiB stride)

Trace facts (NTFF, all 8 cores running):
- 16 SDMA engines per core; engine = outer AP index mod 16, so per-job
  coverage is always an engine PREFIX 0..nr-1 (per-engine loads must be
  non-increasing in engine index).
- Steady state ~318 GB/s payload/core (~20 GB/s/engine HBM->HBM with all
  8 cores active, the HBM wall); packets run ~155ns (26 GB/s) only when
  contention eases.
- Engine 15 is intermittently ~1.28x slower (trn2 quirk), so it gets a
  reduced row count (51 vs 65) sized to finish just-in-time when slow.
- ~7us of NEFF preamble (entry barriers, DMA-table loads, reg init) and
  ~2us completion tail are compiler/runtime-emitted; the reg-init and
  const-memset instructions and the exit barrier are stripped below to
  shrink the critical path and iram image.
"""

import numpy as np

_B_PER_CORE = 4
_N_CORES = 8
_IN_SHAPE = (_B_PER_CORE, 256, 256, 32)
_OUT_SHAPE = (_B_PER_CORE, 8, 8, 32768)
_BAND = 32 * 256 * 32     # elements per (example, row-band)  (262144)

_CACHE = {}


def build_nc():
    import concourse.bass as bass
    import concourse.mybir as mybir

    # Bass.__init__ ends with an all_engine_barrier that makes the DMA
    # sequencers (SP/ACT) wait for GpSimd's slow boot before the first
    # dma_start. Nothing in this kernel reads the init-preamble state,
    # so suppress that barrier (patch scoped to construction).
    orig_barrier = bass.Bass.all_engine_barrier
    bass.Bass.all_engine_barrier = lambda self, **kw: None
    try:
        nc = bass.Bass(
            target_bir_lowering=False,
            enable_partition_id=False,
            monotonic_sem_count=0,
        )
    finally:
        bass.Bass.all_engine_barrier = orig_barrier
    x = nc.dram_tensor("x", list(_IN_SHAPE), mybir.dt.float32, kind="ExternalInput")
    y = nc.dram_tensor("y", list(_OUT_SHAPE), mybir.dt.float32, kind="ExternalOutput")

    # ---- job list ----------------------------------------------------
    # Work unit: one row (band g, row r) = 32 KiB = 8 packets. 32 bands
    # x 32 rows = 1024 rows. Job types (engine = outer index):
    #   ("rows", g, r0, nr)        rows r0..r0+nr-1 of band g -> eng 0..nr-1
    #   ("cols", g, r0, j0, k)     chunks j0..j0+k-1 of rows r0..r0+15 of
    #                              band g -> eng 0..15, k packets each
    #   ("orph", g0, cnt)          row 31 of bands g0..g0+cnt-1 -> eng 0..cnt-1
    #
    # Per-engine row totals: eng0-12: 65, eng13-14: 64, eng15: 51 (=1024).
    # Ramp-in: the first rows0-15 block of each queue is split into
    # column-chunk jobs (16/32/80 descs) so the first doorbell lands
    # early and all 16 engines prime quickly.
    sp_jobs = [("cols", 0, 0, 0, 1), ("cols", 0, 0, 1, 2), ("cols", 0, 0, 3, 5)]
    act_jobs = [("cols", 1, 0, 0, 1), ("cols", 1, 0, 1, 2), ("cols", 1, 0, 3, 5)]

    rest = []
    for g in range(2, 32):
        rest.append(("rows", g, 0, 16))
    for g in range(13, 32):
        rest.append(("rows", g, 16, 16))
    for g in range(0, 13):
        rest.append(("rows", g, 16, 15))
    rest.append(("orph", 0, 13))
    # interleave by descriptor count to keep the two rings balanced
    sp_d = act_d = 128
    for job in rest:
        nd = 8 * job[3] if job[0] == "rows" else 8 * job[2]
        if sp_d <= act_d:
            sp_jobs.append(job)
            sp_d += nd
        else:
            act_jobs.append(job)
            act_d += nd

    def issue(engine, my_jobs, sem):
        n = 0
        for job in my_jobs:
            if job[0] == "rows":
                _, g, r0, nr = job
                off = g * _BAND
                src = bass.AP(
                    x, off + r0 * 8192, [[8192, nr], [1024, 8], [1, 1024]]
                )
                dst = bass.AP(
                    y, off + r0 * 1024, [[1024, nr], [32768, 8], [1, 1024]]
                )
            elif job[0] == "cols":
                _, g, r0, j0, k = job
                off = g * _BAND
                src = bass.AP(
                    x,
                    off + r0 * 8192 + j0 * 1024,
                    [[8192, 16], [1024, k], [1, 1024]],
                )
                dst = bass.AP(
                    y,
                    off + r0 * 1024 + j0 * 32768,
                    [[1024, 16], [32768, k], [1, 1024]],
                )
            else:  # orph: row 31 of cnt consecutive bands
                _, g0, cnt = job
                src = bass.AP(
                    x, g0 * _BAND + 31 * 8192, [[_BAND, cnt], [1024, 8], [1, 1024]]
                )
                dst = bass.AP(
                    y, g0 * _BAND + 31 * 1024, [[_BAND, cnt], [32768, 8], [1, 1024]]
                )
            engine.dma_start(out=dst, in_=src).then_inc(sem, 16)
            n += 16
        if n:
            engine.wait_ge(sem, n)

    with (
        nc.semaphore("sp_sem") as sp_sem,
        nc.semaphore("act_sem") as act_sem,
        nc.Block(no_gpsimd_drain=True) as block,
    ):

        @block.sync
        def _(sync):
            issue(sync, sp_jobs, sp_sem)

        @block.scalar
        def _(scalar):
            issue(scalar, act_jobs, act_sem)

    # ---- strip framework instructions this kernel never consumes -----
    # The bass init emits per-engine register inits (R8=0, R10..13=-1)
    # and const-AP memsets on GpSimd; the Block exit emits drains + a
    # sem-only all-engine barrier. None of that state is read here (no
    # register use, no const APs, engines other than SP/ACT execute
    # nothing), so drop them: SP/ACT reach the first DMA sooner and the
    # unused engines' instruction streams become empty.
    keep_eng = {mybir.EngineType.SP, mybir.EngineType.Activation}
    for bb in nc.main_func.blocks:
        kept = []
        for ins in bb.instructions:
            tn = type(ins).__name__
            if tn in ("InstRegisterMove", "InstMemset"):
                continue
            if tn == "InstDrain" and ins.engine not in keep_eng:
                continue
            if tn == "InstEventSemaphore" and ins.name.startswith("aeb_barrier"):
                continue
            kept.append(ins)
        bb.instructions = kept

    return nc


def _get_nc():
    if "nc" not in _CACHE:
        _CACHE["nc"] = build_nc()
    return _CACHE["nc"]


def kernel(inputs: np.ndarray) -> np.ndarray:
    from concourse.bass_utils import run_bass_kernel_spmd

    inputs = np.ascontiguousarray(np.asarray(inputs, dtype=np.float32))
    assert inputs.shape == (_B_PER_CORE * _N_CORES,) + _IN_SHAPE[1:]

    nc = _get_nc()
    in_maps = [
        {"x": np.ascontiguousarray(inputs[c * _B_PER_CORE : (c + 1) * _B_PER_CORE])}
        for c in range(_N_CORES)
    ]
    res = run_bass_kernel_spmd(nc, in_maps, core_ids=list(range(_N_CORES)))
    return np.concatenate([r["y"] for r in res.results], axis=0)


# revision 8
# speedup vs baseline: 1.0282x; 1.0282x over previous
"""Space-to-depth (8x8 chessboard) kernel for Trainium2.

Full input  : (32, 256, 256, 32) f32
Full output : (32, 8, 8, 32768) f32
out[b, i, j] = inputs[b, i*32:(i+1)*32, j*32:(j+1)*32, :].reshape(-1)

Sharding: batch dim (32) split across 8 NeuronCores (pure data parallel,
no communication) -> 4 examples per core.

Per core the op is pure HBM->HBM data movement, done entirely with DMA
access patterns (no compute engines). Within one (example b, 32-row
band i), iterating (r, j, elem) makes the source AP contiguous and the
destination a 3D AP, so a single DMA moves a block of rows in 4 KiB
contiguous chunks:

  src [[8192, nr], [1024, k], [1, 1024]]   (contiguous 32 KiB per row r)
  dst [[1024, nr], [32768, k], [1, 1024]]  (4 KiB chunks, 32 KiB stride)

This variant issues the whole stream from the SP (sync) HWDGE queue
only: HWDGE descgen capability is ~6ns/desc (measured from unblocked
DMA_DIRECT2D durations), far above the stream's 12.9ns/desc drain, and
leaving the ACT sequencer untouched may shrink the NEFF static-load
(activation tables) that gates the ~3us entry-barrier wait.
"""

import numpy as np

_B_PER_CORE = 4
_N_CORES = 8
_IN_SHAPE = (_B_PER_CORE, 256, 256, 32)
_OUT_SHAPE = (_B_PER_CORE, 8, 8, 32768)
_BAND = 32 * 256 * 32     # elements per (example, row-band)  (262144)

_CACHE = {}


def build_nc():
    import concourse.bass as bass
    import concourse.mybir as mybir

    orig_barrier = bass.Bass.all_engine_barrier
    bass.Bass.all_engine_barrier = lambda self, **kw: None
    try:
        nc = bass.Bass(
            target_bir_lowering=False,
            enable_partition_id=False,
            monotonic_sem_count=0,
        )
    finally:
        bass.Bass.all_engine_barrier = orig_barrier
    x = nc.dram_tensor("x", list(_IN_SHAPE), mybir.dt.float32, kind="ExternalInput")
    y = nc.dram_tensor("y", list(_OUT_SHAPE), mybir.dt.float32, kind="ExternalOutput")

    # Per-engine row totals: eng0-12: 65, eng13-14: 64, eng15: 51 (=1024;
    # engine 15 hedged for its intermittent ~1.28x slowness). Ramp-in:
    # first block split into column-chunk jobs (16/32/80 descs) so the
    # first doorbell lands early.
    jobs = [("cols", 0, 0, 0, 1), ("cols", 0, 0, 1, 2), ("cols", 0, 0, 3, 5)]
    for g in range(1, 32):
        jobs.append(("rows", g, 0, 16))
    for g in range(13, 32):
        jobs.append(("rows", g, 16, 16))
    for g in range(0, 13):
        jobs.append(("rows", g, 16, 15))
    jobs.append(("orph", 0, 13))

    def issue(engine, my_jobs, sem):
        n = 0
        for job in my_jobs:
            if job[0] == "rows":
                _, g, r0, nr = job
                off = g * _BAND
                src = bass.AP(
                    x, off + r0 * 8192, [[8192, nr], [1024, 8], [1, 1024]]
                )
                dst = bass.AP(
                    y, off + r0 * 1024, [[1024, nr], [32768, 8], [1, 1024]]
                )
            elif job[0] == "cols":
                _, g, r0, j0, k = job
                off = g * _BAND
                src = bass.AP(
                    x,
                    off + r0 * 8192 + j0 * 1024,
                    [[8192, 16], [1024, k], [1, 1024]],
                )
                dst = bass.AP(
                    y,
                    off + r0 * 1024 + j0 * 32768,
                    [[1024, 16], [32768, k], [1, 1024]],
                )
            else:  # orph: row 31 of cnt consecutive bands
                _, g0, cnt = job
                src = bass.AP(
                    x, g0 * _BAND + 31 * 8192, [[_BAND, cnt], [1024, 8], [1, 1024]]
                )
                dst = bass.AP(
                    y, g0 * _BAND + 31 * 1024, [[_BAND, cnt], [32768, 8], [1, 1024]]
                )
            engine.dma_start(out=dst, in_=src).then_inc(sem, 16)
            n += 16
        if n:
            engine.wait_ge(sem, n)

    # sanity: per-engine row totals must be (65*13, 64*2, 51)
    rows = [0] * 16
    for job in jobs:
        if job[0] == "rows":
            for e in range(job[3]):
                rows[e] += 1
        elif job[0] == "cols":
            for e in range(16):
                rows[e] += job[4] / 8.0
        else:
            for e in range(job[2]):
                rows[e] += 1
    assert sum(rows) == 1024, rows
    assert rows == [65] * 13 + [64] * 2 + [51], rows

    with (
        nc.semaphore("sp_sem") as sp_sem,
        nc.Block(no_gpsimd_drain=True) as block,
    ):

        @block.sync
        def _(sync):
            issue(sync, jobs, sp_sem)

    return nc


def _get_nc():
    if "nc" not in _CACHE:
        _CACHE["nc"] = build_nc()
    return _CACHE["nc"]


def kernel(inputs: np.ndarray) -> np.ndarray:
    from concourse.bass_utils import run_bass_kernel_spmd

    inputs = np.ascontiguousarray(np.asarray(inputs, dtype=np.float32))
    assert inputs.shape == (_B_PER_CORE * _N_CORES,) + _IN_SHAPE[1:]

    nc = _get_nc()
    in_maps = [
        {"x": np.ascontiguousarray(inputs[c * _B_PER_CORE : (c + 1) * _B_PER_CORE])}
        for c in range(_N_CORES)
    ]
    res = run_bass_kernel_spmd(nc, in_maps, core_ids=list(range(_N_CORES)))
    return np.concatenate([r["y"] for r in res.results], axis=0)


# revision 10
# speedup vs baseline: 1.0744x; 1.0449x over previous
"""Space-to-depth (8x8 chessboard) kernel for Trainium2.

Full input  : (32, 256, 256, 32) f32
Full output : (32, 8, 8, 32768) f32
out[b, i, j] = inputs[b, i*32:(i+1)*32, j*32:(j+1)*32, :].reshape(-1)

Sharding: batch dim (32) split across 8 NeuronCores (pure data parallel,
no communication) -> 4 examples per core.

Per core the op is pure HBM->HBM data movement, done entirely with DMA
access patterns (no compute engines). Key layout fact: within one
(example b, 32-row band i), iterating (r, j, elem) makes the source AP
contiguous and the destination a 3D AP, so a single DMA moves a
half-band (16 rows = 512 KiB) in 4 KiB contiguous chunks:

  src [[8192, nr], [1024, 8], [1, 1024]]   (contiguous 32 KiB per row r)
  dst [[1024, nr], [32768, 8], [1, 1024]]  (4 KiB chunks, 32 KiB stride)

Performance notes (measured on trn2 via NTFF traces):
- SDMA engine assignment is (outer AP dim index) mod 16, so outer count
  >= 16 engages all 16 SDMA engines (outer 8 uses only engines 0-7).
- Keep HWDGE DMAs at <= 128 descriptors (outer <= 16): outer 31/32 DMAs
  hit a slow descriptor-generation fallback that blocks the issuing
  sequencer 10-100 us per instruction and starves the engines (6x slower).
- Issuing from both HWDGE queues (sync=SP + scalar=ACT) beats one queue;
  a third stream via gpsimd SWDGE is a net loss (SBUF descriptor-ring
  port contention slows the HWDGE streams, 137 us).
- ~315-320 GB/s payload per core is a hard wall: a plain contiguous
  HBM->HBM copy measures the same 114-117 us regardless of descriptor
  size (2-32 KiB sweep), i.e. 16 SDMA engines x ~20.6 GB/s sustained
  (the per-NC HBM limit). The permutation itself is free; the only
  tunables left are schedule shape: engine load balance and where the
  uneven jobs sit in the stream.
- Fixed runtime behaviors (every NEFF, every geometry): ~5.2 us preamble
  before the first descriptor moves, engines 5-15 receive their first
  descriptors ~3.5 us after engines 0-4, ~2 us completion tail.
- SDMA engine 15 is intermittently ~1.25x slower on this dst pattern
  (known trn2 quirk; absent in contiguous copies), so the job list is
  skewed: 16 of the 64 half-band DMAs carry 15 rows instead of 16
  (their unit 15 would land on engine 15), and the 16 skipped rows are
  covered by two batched "orphan" DMAs (outer 8 -> engines 0-7, which
  also soaks up engines 0-4's ~3.5 us head start). The orphans are
  issued early: issuing them last serializes a ~2.5 us tail onto
  engines 0-7 while 8-15 sit idle (costs ~1 us).

Steady state ~310 GB/s payload (~620 GB/s HBM read+write traffic per
core) with all 8 cores running, equal to a plain contiguous HBM->HBM
copy - the kernel runs at the achievable DMA/HBM roofline. HW exec
~114-115 us per core (schedule variants measure reproducibly within
+-0.5 us; alternatives tried: orphans-last 115-116, flat 520-desc
balance 117.6, skew-12 116.6, outer-16 orphan 121, SWDGE-only 133).

Second optimization session (same-day baseline re-measured 115.8; the
device drifts +-1-2 us between days) confirmed the wall and added:
- exec_time counts the walrus-prepended preamble (~6.7 us: entry
  barrier gated on a ~3 us static-DMA load of the ~70 KB NEFF data,
  DMA-table reg loads, reg init) and the ~1.9 us sem-receipt tail, but
  NOT the post-exit-barrier cleanup. Stripping the Block exit barrier
  backfires badly (cleanup enters the measured window: 124 us).
- enable_partition_id=False does not remove the R130/131 loads (they
  are the DMA queue table, not partition id).
- Single-queue (SP-only) streams at ~208-212 ns/packet vs ~200 for two
  queues (120.7 us total) and does not shrink the static load.
- Flat per-engine balance (65/65/.../49 rows) + small first jobs to
  pull the first doorbell earlier measured a wash (115.9) - per-engine
  pace is HBM-supply-limited (median packet 158 ns vs 200 ns pace), so
  idle-engine capacity is partially reabsorbed by the survivors either
  way. HWDGE descgen is ~5.7 ns/desc unblocked - not a bottleneck.
"""

import numpy as np

_B_PER_CORE = 4
_N_CORES = 8
_IN_SHAPE = (_B_PER_CORE, 256, 256, 32)
_OUT_SHAPE = (_B_PER_CORE, 8, 8, 32768)
_EX = 256 * 256 * 32      # elements per example  (2097152)
_BAND = 32 * 256 * 32     # elements per (example, row-band)  (262144)

_CACHE = {}


def build_nc():
    import concourse.bass as bass
    import concourse.mybir as mybir

    # Bass.__init__ ends with an all_engine_barrier that makes the DMA
    # sequencers (SP/ACT) wait for GpSimd's slow boot before the first
    # dma_start, costing ~1-3 us of ramp. Nothing in this kernel reads
    # the init-preamble state (const SBUF tensors / gpsimd), so suppress
    # that one barrier. The patch is scoped to construction and restored
    # before returning; the Block exit barrier is emitted normally.
    orig_barrier = bass.Bass.all_engine_barrier
    bass.Bass.all_engine_barrier = lambda self, **kw: None
    try:
        nc = bass.Bass(target_bir_lowering=False)
    finally:
        bass.Bass.all_engine_barrier = orig_barrier
    x = nc.dram_tensor("x", list(_IN_SHAPE), mybir.dt.float32, kind="ExternalInput")
    y = nc.dram_tensor("y", list(_OUT_SHAPE), mybir.dt.float32, kind="ExternalOutput")

    # Job list: half-band DMAs keyed by global band g = 8*b + i (bands
    # are contiguous across examples, stride _BAND).  The h=1 jobs of
    # bands g<16 carry 15 rows (engine-15 skew); their skipped row 31 is
    # covered by the two orphan DMAs, issued early (positions 2-3) so
    # their engine-0-7-only load lands mid-stream instead of serializing
    # the kernel tail.  Resulting per-engine loads: eng0-7: 528 descs,
    # eng8-14: 512, eng15: 384 - co-terminating given engines 0-4's
    # ~3.5 us earlier start and engine 15's intermittent 1.25x slowness.
    jobs = [
        (g, h * 16, 15 if (h == 1 and g < 16) else 16)
        for g in range(32)
        for h in range(2)
    ]
    jobs = jobs[:2] + [("orph", 0, 8), ("orph", 8, 8)] + jobs[2:]

    def issue(engine, my_jobs, sem):
        n = 0
        for job in my_jobs:
            if job[0] == "orph":
                # rows r=31 of `cnt` consecutive bands starting at g0;
                # one 32 KiB unit per band -> SDMA engines 0..cnt-1
                _, g0, cnt = job
                src = bass.AP(
                    x, g0 * _BAND + 31 * 8192, [[_BAND, cnt], [1024, 8], [1, 1024]]
                )
                dst = bass.AP(
                    y, g0 * _BAND + 31 * 1024, [[_BAND, cnt], [32768, 8], [1, 1024]]
                )
            else:
                g, r0, nr = job
                off = g * _BAND
                src = bass.AP(
                    x, off + r0 * 8192, [[8192, nr], [1024, 8], [1, 1024]]
                )
                dst = bass.AP(
                    y, off + r0 * 1024, [[1024, nr], [32768, 8], [1, 1024]]
                )
            engine.dma_start(out=dst, in_=src).then_inc(sem, 16)
            n += 16
        if n:
            engine.wait_ge(sem, n)

    with (
        nc.semaphore("sp_sem") as sp_sem,
        nc.semaphore("act_sem") as act_sem,
        nc.Block(no_gpsimd_drain=True) as block,
    ):

        @block.sync
        def _(sync):
            issue(sync, jobs[0::2], sp_sem)

        @block.scalar
        def _(scalar):
            issue(scalar, jobs[1::2], act_sem)

    return nc


def _get_nc():
    if "nc" not in _CACHE:
        _CACHE["nc"] = build_nc()
    return _CACHE["nc"]


def kernel(inputs: np.ndarray) -> np.ndarray:
    from concourse.bass_utils import run_bass_kernel_spmd

    inputs = np.ascontiguousarray(np.asarray(inputs, dtype=np.float32))
    assert inputs.shape == (_B_PER_CORE * _N_CORES,) + _IN_SHAPE[1:]

    nc = _get_nc()
    in_maps = [
        {"x": np.ascontiguousarray(inputs[c * _B_PER_CORE : (c + 1) * _B_PER_CORE])}
        for c in range(_N_CORES)
    ]
    res = run_bass_kernel_spmd(nc, in_maps, core_ids=list(range(_N_CORES)))
    return np.concatenate([r["y"] for r in res.results], axis=0)

